# revision 27
# baseline (speedup 1.0000x reference)
"""DinkNet GNN (2-layer GraphConv encoder, two views) on 8 Trainium2 NeuronCores.

Strategy (self-contained; shapes hardcoded for the nn_DinkNet_dgl problem):
  - Nodes are sharded across 8 cores; per-core node u (sorted position) maps to
    table row c*NLP + (u%128)*T + (u//128) so per-partition slab writes are
    contiguous. Graph preprocessing (balance, rounds, index arrays) on host.
  - Math folding: (x*no) @ W1 = no * (x@W1); encode(x[perm]) reuses y = x@W1
    via row gathers; final (z@Wm+bm).sum(1) = h2 @ Wm.sum(1) + bm.sum().
  - The segment-sums over 800k edges run as batched GPSIMD dma_gather rounds
    (int16 indices; node space split in two halves so local indices fit).
    Both views interleave in one 512B table row (one gather feeds both views).
  - dma_gather descriptor emission on the Q7 pair is the bottleneck
    (~8ns/idx), so padding is minimized: src halves are balanced per-dst by
    local search, and nodes are tiled by max(c0,c1) so per-tile slot counts
    are tight. Padding indices cycle over 88 distinct zero rows to avoid
    serializing the HBM bank holding a single dummy row.
"""
import os
import numpy as np
import ml_dtypes

import concourse.bass as bass
import concourse.bacc as bacc
import concourse.mybir as mybir
import concourse.tile as tile
from concourse.bass_utils import run_bass_kernel_spmd
from concourse.masks import make_identity

# Problem shapes (hardcoded per contract).
N, E, FIN, H = 50000, 800000, 500, 96
C = 8                 # cores
P = 128               # partitions
NL = N // C           # 6250 real nodes per core
T = (NL + P - 1) // P # 49 rank tiles per core
NLP = T * P           # 6272 padded nodes per core
NTAB = C * NLP        # 50176 table rows
HB = NTAB // 2        # 25088: half boundary for int16 indices
EL = 128              # padded per-view row length (bf16 -> 256B)
EL2 = 2 * EL          # pair row: [view1 | view2] -> 512B in bf16
KS = 4                # k-slabs for the x @ W1 matmul (500 = 4*125)
KSL = FIN // KS       # 125
NB = 7                # tiles per batched x load (49 = 7*7)
RBCAP = T             # max pair-blocks per merged gather (SBUF budget)

F32 = mybir.dt.float32
BF16 = mybir.dt.bfloat16
TB = BF16
I16 = mybir.dt.int16
AOp = mybir.AluOpType

_cache = {}


def _bc_inner(ap, n):
    """Broadcast AP over a new innermost dim of size n (stride 0)."""
    return bass.AP(ap.tensor, ap.offset, list(ap.ap) + [[0, n]])


def _bc_middle(ap2d, n):
    """[128, F] AP -> [128, n, F] with the middle dim broadcast (stride 0)."""
    a = list(ap2d.ap)
    return bass.AP(ap2d.tensor, ap2d.offset, [a[0], [0, n], a[1]])


def _build(rounds, hasb1, hasb2, iw16, pw16, sim_mode=False):
    """rounds: list of (half, [tn...], off16) shared by both SpMM layers."""
    nc = bacc.Bacc(None, num_devices=1 if sim_mode else C,
                   target_bir_lowering=False, debug=False,
                   num_swdge_queues=4)

    # ---- inputs ----
    xtb = nc.dram_tensor("xtb", [NB, KSL, NB * KS * P], BF16, kind="ExternalInput")
    xpb = nc.dram_tensor("xpb", [NB, KSL, NB * KS * P], BF16, kind="ExternalInput")
    idx = nc.dram_tensor("idx", [P, iw16], I16, kind="ExternalInput")
    no_in = nc.dram_tensor("no_in", [P, T], F32, kind="ExternalInput")
    ni_in = nc.dram_tensor("ni_in", [P, T], F32, kind="ExternalInput")
    w1_in = nc.dram_tensor("w1_in", [KS, KSL, H], BF16, kind="ExternalInput")
    w2_in = nc.dram_tensor("w2_in", [H, H], BF16, kind="ExternalInput")
    b1_in = nc.dram_tensor("b1_in", [P, H], F32, kind="ExternalInput")
    a1_in = nc.dram_tensor("a1_in", [P, H], F32, kind="ExternalInput")
    b2_in = nc.dram_tensor("b2_in", [P, H], F32, kind="ExternalInput")
    a2_in = nc.dram_tensor("a2_in", [P, H], F32, kind="ExternalInput")
    wv_in = nc.dram_tensor("wv_in", [P, H + 1], F32, kind="ExternalInput")
    out = nc.dram_tensor("out", [2, NLP], F32, kind="ExternalOutput")

    nphase = int(os.environ.get("KERNEL_NPHASE", "9"))

    with tile.TileContext(nc) as tc:
        with (
            tc.tile_pool(name="cst", bufs=1) as cst,
            tc.tile_pool(name="wrk", bufs=2) as wrk,
            tc.tile_pool(name="ps", bufs=2, space="PSUM") as ps,
            tc.tile_pool(name="dr", bufs=1, space="DRAM") as dr,
        ):
            # ---- resident constants ----
            no_t = cst.tile([P, T], F32)
            nc.sync.dma_start(out=no_t[:], in_=no_in[:])
            ni_t = cst.tile([P, T], F32)
            nc.sync.dma_start(out=ni_t[:], in_=ni_in[:])
            w1t = cst.tile([KSL, KS * H], BF16)
            nc.sync.dma_start(
                out=w1t[:].rearrange("p (s h) -> p s h", s=KS),
                in_=w1_in[:].rearrange("s p h -> p s h"),
            )
            w2t = cst.tile([H, H], BF16)
            nc.sync.dma_start(out=w2t[:], in_=w2_in[:])
            b1r = cst.tile([P, H], F32)
            nc.sync.dma_start(out=b1r[:], in_=b1_in[:])
            a1r = cst.tile([P, H], F32)
            nc.sync.dma_start(out=a1r[:], in_=a1_in[:])
            b2r = cst.tile([P, H], F32)
            nc.sync.dma_start(out=b2r[:], in_=b2_in[:])
            a2r = cst.tile([P, H], F32)
            nc.sync.dma_start(out=a2r[:], in_=a2_in[:])
            wvr = cst.tile([P, H + 1], F32)
            nc.sync.dma_start(out=wvr[:], in_=wv_in[:])
            identb = cst.tile([P, P], TB)
            make_identity(nc, identb[:])

            # Pool touches: advance Pool's vector clock past the const loads
            tch = cst.tile([1, 8], I16, name="tch")
            nc.gpsimd.dma_start(out=tch[0:1, 0:1], in_=no_t[0:1, 0:1])

            # ---- DRAM tables ----
            ypair = dr.tile([NTAB, EL2], TB, addr_space="Shared", name="ypair")
            gpair = dr.tile([NTAB, EL2], TB, addr_space="Shared", name="gpair")
            psl = dr.tile([NLP, EL2], TB, name="psl")
            gsl = dr.tile([NLP, EL2], TB, name="gsl")

            tchf = cst.tile([1, 4], TB, name="tchf")

            def allgather(sl, tab):
                if sim_mode:
                    nc.sync.dma_start(out=tab[0:NLP, :], in_=sl[:])
                else:
                    nc.gpsimd.collective_compute(
                        "AllGather", AOp.bypass,
                        replica_groups=[list(range(C))],
                        ins=[sl[:].opt()], outs=[tab[:].opt()],
                    )
                nc.gpsimd.dma_start(out=tchf[:], in_=tab[0:1, 0:4])

            # ---- phase 1: both views y_v = no * (x_v @ W1), v in {orig, perm} ----
            # pstage holds pair rows at EL2 pitch (pad cols pre-zeroed) so the
            # psl slab write is one per-partition-contiguous DMA. The permuted
            # view's x rows are pre-gathered on the host (perm is host data).
            pstage = wrk.tile([P, T * EL2], TB, tag="pstage", bufs=1)
            nc.vector.memset(pstage[:], 0.0)
            for b in range(NB):
                xtile = wrk.tile([KSL, NB * KS * P], BF16, tag="xtile", bufs=2)
                nc.sync.dma_start(out=xtile[:], in_=xtb[b])
                xptile = wrk.tile([KSL, NB * KS * P], BF16, tag="xtile", bufs=2)
                nc.sync.dma_start(out=xptile[:], in_=xpb[b])
                # 4 PSUM banks per view batch -> one strided DVE drain each
                for t0 in range(0, NB, 4):
                    ntt = min(4, NB - t0)
                    for v, xt_ in enumerate((xtile, xptile)):
                        yps = ps.tile([P, 4 * H], F32, tag="bank", bufs=8, space="PSUM")
                        for q in range(ntt):
                            tt = t0 + q
                            for s in range(KS):
                                nc.tensor.matmul(
                                    out=yps[:, q * H:(q + 1) * H],
                                    lhsT=xt_[:, (tt * KS + s) * P:(tt * KS + s + 1) * P],
                                    rhs=w1t[:, s * H:(s + 1) * H],
                                    start=(s == 0), stop=(s == KS - 1),
                                )
                        t = b * NB + t0
                        ydst = bass.AP(pstage.tensor,
                                       pstage.offset + t * EL2 + v * EL,
                                       [list(pstage.ap[0]), [EL2, ntt], [1, H]])
                        nc.vector.tensor_tensor(
                            out=ydst,
                            in0=yps[:, :ntt * H].rearrange("p (n h) -> p n h", h=H),
                            in1=_bc_inner(no_t[:, t:t + ntt], H), op=AOp.mult,
                        )
            if nphase >= 3:
                nc.sync.dma_start(
                    out=bass.AP(psl.tensor, psl.offset, [[T * EL2, P], [1, T * EL2]]),
                    in_=pstage[:],
                )
                allgather(psl, ypair)

            # ---- SpMM over a pair table; both views accumulated ----
            # Tiles 0..TP-1 accumulate on the TensorEngine (identity matmul
            # into PSUM, bank-chunked 5 tiles each); tiles TP.. on DVE.
            TP = 20
            NCHUNK = TP // 5
            # flat sub-round list to place start/stop flags per PSUM chunk
            subs = []
            for (h, tns, off16) in rounds:
                for si, tn in enumerate(tns):
                    subs.append(tn)
            last_cover = {}
            for k in range(NCHUNK):
                last_cover[k] = max(i for i, tn in enumerate(subs) if tn > 5 * k)

            def spmm_pair(tab, layer, br, ar, hasb):
                accs = []
                for v in range(2):
                    acc = wrk.tile([P, T * H], F32, tag=f"acc{v}", bufs=1,
                                   name=f"acc{v}_{layer}")
                    nc.vector.memset(acc[:, TP * H:], 0.0)
                    accs.append(acc)
                accps = [[ps.tile([P, 5 * H], F32, tag="bank",
                                  space="PSUM", bufs=8, name=f"accps{v}_{k}_{layer}")
                          for k in range(NCHUNK)] for v in range(2)]
                si = 0
                for ri, (h, tns, off16) in enumerate(rounds):
                    ntot = sum(tns)
                    rb = wrk.tile([P, RBCAP * EL2], TB, tag="rb2", bufs=4,
                                  name=f"rb_{layer}_{h}_{off16}")
                    ixw = ntot * P // 16
                    ixt = wrk.tile([P, RBCAP * P // 16], I16, tag="idxs", bufs=2,
                                   name=f"ix_{layer}_{h}_{off16}")
                    nc.sync.dma_start(out=ixt[:, :ixw],
                                      in_=idx[:, off16:off16 + ixw])
                    src = tab[0:HB, :] if h == 0 else tab[HB:NTAB, :]
                    nc.gpsimd.dma_gather(
                        out_ap=rb[:, :ntot * EL2].rearrange("p (b e) -> p b e", e=EL2),
                        in_ap=src,
                        idxs_ap=ixt[:, :ixw],
                        num_idxs=ntot * P, num_idxs_reg=ntot * P, elem_size=EL2,
                        single_packet=False, queue_num=ri % 4,
                    )
                    boff = 0
                    for tn in tns:
                        for v in range(2):
                            # PE part: tiles 0..min(tn,TP)
                            for k in range(NCHUNK):
                                nb = min(tn, TP) - 5 * k
                                if nb <= 0:
                                    break
                                nb = min(nb, 5)
                                r3 = bass.AP(
                                    rb.tensor,
                                    rb.offset + (boff + 5 * k) * EL2 + v * EL,
                                    [list(rb.ap[0]), [EL2, nb], [1, H]],
                                )
                                nc.tensor.matmul(
                                    out=accps[v][k][:, :nb * H],
                                    lhsT=identb[:], rhs=r3,
                                    start=(si == 0), stop=(si == last_cover[k]),
                                )
                            # DVE tail: tiles TP..tn
                            if tn > TP:
                                a3 = bass.AP(
                                    accs[v].tensor,
                                    accs[v].offset + TP * H,
                                    [list(accs[v].ap[0]), [H, tn - TP], [1, H]],
                                )
                                r3t = bass.AP(
                                    rb.tensor,
                                    rb.offset + (boff + TP) * EL2 + v * EL,
                                    [list(rb.ap[0]), [EL2, tn - TP], [1, H]],
                                )
                                nc.vector.tensor_tensor(out=a3, in0=a3, in1=r3t, op=AOp.add)
                        boff += tn
                        si += 1
                # drain PSUM chunks into acc (add onto memset-zero region)
                for v in range(2):
                    for k in range(NCHUNK):
                        nc.vector.tensor_copy(
                            out=accs[v][:, 5 * k * H:(5 * k + 5) * H],
                            in_=accps[v][k][:],
                        )
                # postops per view (bf16 for 2x DVE rate; acc stays f32)
                outs_v = []
                for v in range(2):
                    acc = accs[v]
                    a3 = acc[:].rearrange("p (t h) -> p t h", h=H)
                    hb = wrk.tile([P, T * H], TB, tag=f"hb{v}", bufs=1,
                                  name=f"hb_{layer}_{v}")
                    h3 = hb[:].rearrange("p (t h) -> p t h", h=H)
                    nc.vector.tensor_tensor(out=h3, in0=a3, in1=_bc_inner(ni_t[:], H), op=AOp.mult)
                    if hasb:
                        nc.vector.tensor_tensor(out=h3, in0=h3, in1=_bc_middle(br[:], T), op=AOp.add)
                    pos = wrk.tile([P, T * H], TB, tag="pos", bufs=1,
                                   name=f"pos_{layer}_{v}")
                    nc.vector.tensor_scalar(out=pos[:], in0=hb[:], scalar1=0.0, scalar2=None, op0=AOp.max)
                    nc.vector.tensor_scalar(out=hb[:], in0=hb[:], scalar1=0.0, scalar2=None, op0=AOp.min)
                    nc.vector.tensor_tensor(out=h3, in0=h3, in1=_bc_middle(ar[:], T), op=AOp.mult)
                    nc.vector.tensor_tensor(out=hb[:], in0=hb[:], in1=pos[:], op=AOp.add)
                    if layer == 1:
                        nc.vector.tensor_tensor(out=h3, in0=h3, in1=_bc_inner(no_t[:], H), op=AOp.mult)
                        gst = wrk.tile([P, T * EL2], TB, tag="pstage", bufs=1,
                                       name="gstage") if v == 0 else outs_v[0]
                        if v == 0:
                            nc.vector.memset(gst[:], 0.0)
                        # batched: 4 transposes into one PSUM bank, one copy,
                        # 4 matmuls into one bank, one strided copy out
                        for t0 in range(0, T, 4):
                            nt = min(4, T - t0)
                            tp = ps.tile([H, 4 * P], TB, tag="bank", bufs=8, space="PSUM")
                            for q in range(nt):
                                nc.tensor.transpose(
                                    out=tp[:, q * P:(q + 1) * P],
                                    in_=hb[:, (t0 + q) * H:(t0 + q + 1) * H],
                                    identity=identb[:],
                                )
                            gsT = wrk.tile([H, 4 * P], TB, tag="gsT", bufs=1)
                            nc.vector.tensor_copy(out=gsT[:, :nt * P], in_=tp[:, :nt * P])
                            gp = ps.tile([P, 4 * H], F32, tag="bank", bufs=8, space="PSUM")
                            for q in range(nt):
                                nc.tensor.matmul(out=gp[:, q * H:(q + 1) * H],
                                                 lhsT=gsT[:, q * P:(q + 1) * P],
                                                 rhs=w2t[:], start=True, stop=True)
                            gdst = bass.AP(gst.tensor, gst.offset + t0 * EL2 + v * EL,
                                           [list(gst.ap[0]), [EL2, nt], [1, H]])
                            nc.vector.tensor_copy(
                                out=gdst,
                                in_=gp[:, :nt * H].rearrange("p (n h) -> p n h", h=H))
                        outs_v.append(gst)
                    else:
                        zb = wrk.tile([P, T], F32, tag="zb", bufs=1, name=f"zb_{v}")
                        nc.vector.tensor_tensor(out=h3, in0=h3,
                                                in1=_bc_middle(wvr[:, 0:H], T), op=AOp.mult)
                        nc.vector.tensor_reduce(
                            out=zb[:], in_=h3, axis=mybir.AxisListType.X, op=AOp.add,
                        )
                        nc.vector.tensor_scalar(
                            out=zb[:], in0=zb[:], scalar1=wvr[:, H:H + 1],
                            scalar2=None, op0=AOp.add,
                        )
                        nc.sync.dma_start(
                            out=bass.AP(out, v * NLP, [[T, P], [1, T]]), in_=zb[:],
                        )
                        outs_v.append(None)
                return outs_v

            if nphase >= 4:
                g1, g2 = spmm_pair(ypair, 1, b1r, a1r, hasb1)
                nc.sync.dma_start(
                    out=bass.AP(gsl.tensor, gsl.offset, [[T * EL2, P], [1, T * EL2]]),
                    in_=g2[:],
                )
                allgather(gsl, gpair)
            if nphase >= 5:
                spmm_pair(gpair, 2, b2r, a2r, hasb2)

    nc.finalize()
    return nc


def _wrap16(vals):
    """[n] int array -> wrapped [128, n/16] int16 (replicated across 8 Q7 cores)."""
    n = vals.shape[-1]
    assert n % 16 == 0
    w = vals.reshape(n // 16, 16).T.astype(np.int16)
    return np.tile(w, (8, 1))


def _balance_halves(src, dst, scnt, sstart, dst_by_src):
    """Greedy + local-search 2-coloring of src nodes so each dst's in-edges
    split evenly between halves. Returns half_of (int8[N])."""
    cnt_diff = np.zeros(N, np.int32)
    half_of = np.zeros(N, np.int8)
    cap = [N // 2, N // 2]
    order = np.argsort(-scnt, kind="stable")
    for s_ in order:
        D = dst_by_src[sstart[s_]:sstart[s_ + 1]]
        sd = cnt_diff[D].sum() if len(D) else (cap[1] - cap[0])
        h = 0 if sd < 0 else 1
        if cap[h] == 0:
            h = 1 - h
        half_of[s_] = h
        cap[h] -= 1
        if len(D):
            cnt_diff[D] += 1 - 2 * h

    sizes = np.bincount(half_of, minlength=2)
    for sweep in range(6):
        nflip = 0
        order = np.random.RandomState(sweep).permutation(N)
        for s_ in order:
            b, e_ = sstart[s_], sstart[s_ + 1]
            if b == e_:
                continue
            D = dst_by_src[b:e_]
            sd = int(cnt_diff[D].sum())
            k = e_ - b
            h = half_of[s_]
            if h == 0 and sd > k and sizes[1] < N // 2 + 64:
                half_of[s_] = 1; cnt_diff[D] -= 2
                sizes[0] -= 1; sizes[1] += 1; nflip += 1
            elif h == 1 and -sd > k and sizes[0] < N // 2 + 64:
                half_of[s_] = 0; cnt_diff[D] += 2
                sizes[1] -= 1; sizes[0] += 1; nflip += 1
        if nflip < 50:
            break
    imbal = int(sizes[0]) - N // 2
    if imbal != 0:
        h_from = 0 if imbal > 0 else 1
        cands = np.where(half_of == h_from)[0]
        damage = np.zeros(len(cands))
        for i, s_ in enumerate(cands):
            D = dst_by_src[sstart[s_]:sstart[s_ + 1]]
            sd = int(cnt_diff[D].sum())
            k = len(D)
            damage[i] = (4 * k - 4 * sd) if h_from == 0 else (4 * k + 4 * sd)
        for s_ in cands[np.argsort(damage)[:abs(imbal)]]:
            D = dst_by_src[sstart[s_]:sstart[s_ + 1]]
            half_of[s_] = 1 - h_from
            cnt_diff[D] += -2 if h_from == 0 else 2
    return half_of


def kernel(x, src, dst, perm, W1, b1, a1, W2, b2, a2, Wm, bm):
    x = np.ascontiguousarray(np.asarray(x, np.float32))
    src = np.asarray(src, np.int64)
    dst = np.asarray(dst, np.int64)
    perm = np.asarray(perm, np.int64)
    W1 = np.asarray(W1, np.float32); W2 = np.asarray(W2, np.float32)
    Wm = np.asarray(Wm, np.float32)
    b1 = np.asarray(b1, np.float32); b2 = np.asarray(b2, np.float32)
    a1 = np.asarray(a1, np.float32); a2 = np.asarray(a2, np.float32)
    bm = np.asarray(bm, np.float32)

    deg_out = np.bincount(src, minlength=N)
    deg_in = np.bincount(dst, minlength=N)
    norm_out = np.maximum(deg_out, 1).astype(np.float32) ** -0.5
    norm_in = np.maximum(deg_in, 1).astype(np.float32) ** -0.5

    eo = np.argsort(src, kind="stable")
    dst_by_src = dst[eo]
    scnt = np.bincount(src, minlength=N)
    sstart = np.concatenate(([0], np.cumsum(scnt)))
    half_of = _balance_halves(src, dst, scnt, sstart, dst_by_src)

    # per-half in-edge counts of every dst
    c0 = np.bincount(dst[half_of[src] == 0], minlength=N)
    c1 = np.bincount(dst[half_of[src] == 1], minlength=N)
    cmax = np.maximum(c0, c1); cmin = np.minimum(c0, c1)

    # per-half sort by max(c0,c1) so tiles have tight slot counts; stripe
    # over 4 cores each. ranked[c] lists nodes in position order u; the table
    # row of position u is (u%128)*T + u//128 (partition-contiguous slabs).
    ranked = np.empty((C, NL), np.int64)
    Lmap = np.empty(N, np.int64)   # node -> table row
    Umap = np.empty(N, np.int64)   # node -> core-local position u
    Cmap = np.empty(N, np.int64)   # node -> core
    for h in range(2):
        ids = np.where(half_of == h)[0]
        ids = ids[np.lexsort((-cmin[ids], -cmax[ids]))]
        for cc in range(4):
            c = h * 4 + cc
            rid = ids[cc::4]
            ranked[c] = rid
            u = np.arange(NL)
            Umap[rid] = u
            Cmap[rid] = c
            Lmap[rid] = c * NLP + (u % P) * T + (u // P)

    lsrc = Lmap[src]
    half = (lsrc >= HB).astype(np.int64)

    # dst-side positions in u space
    udst = Umap[dst]
    cdst = Cmap[dst]
    key = (cdst * NLP + udst) * 2 + half
    es = np.argsort(key, kind="stable")
    key_s = key[es]
    cnt = np.bincount(key_s, minlength=2 * NTAB)
    starts = np.concatenate(([0], np.cumsum(cnt)))[:-1]
    slot = np.arange(E) - starts[key_s]
    lsrc_s = lsrc[es]
    half_s = half[es]

    c_e = cdst[es]
    r_e = udst[es]          # position within core (u space)
    t_e = r_e // P
    kh = np.zeros((2, C, T), np.int64)
    np.maximum.at(kh, (half_s, c_e, t_e), slot + 1)
    kcom = kh.max(axis=1)
    kcom = np.maximum.accumulate(kcom[:, ::-1], axis=1)[:, ::-1]

    # merged rounds, capped at RBCAP pair-blocks per gather
    rounds = []
    col16 = 0
    round_off = {}
    for h in range(2):
        j = 0
        kmax = int(kcom[h, 0])
        while j < kmax:
            tns = []
            while j < kmax:
                tn = int((kcom[h] > j).sum())
                if tns and sum(tns) + tn > RBCAP:
                    break
                round_off[(h, j)] = col16 + sum(tns) * P // 16
                tns.append(tn)
                j += 1
            rounds.append((h, tns, col16))
            col16 += sum(tns) * P // 16
    iw16 = col16

    # padding indices cycle over the 88 zero rows of each half (distinct HBM
    # addresses -> no single-bank serialization). Zero rows per half: cores'
    # positions u in [NL, NLP) -> rows (u%128)*T + 48.
    zrows = np.empty((2, 4 * (NLP - NL)), np.int64)
    for h in range(2):
        rows = []
        for u in range(NL, NLP):
            for cc in range(4):
                rows.append(cc * NLP + (u % P) * T + (u // P))
        zrows[h] = np.array(rows, np.int64)
    NZ = zrows.shape[1]

    IDX = np.empty((C, iw16, 16), np.int16)
    for c in range(C):
        # fill with cycling zero-row pattern per half section
        fill = np.empty(iw16 * 16, np.int64)
        pos = np.arange(iw16 * 16)
        fill[:] = zrows[0][(pos + c * 7) % NZ]
        IDX[c] = fill.reshape(iw16, 16)
    # overwrite half-1 sections with half-1 zero rows
    h1cols = []
    for (h, tns, off16) in rounds:
        if h == 1:
            h1cols.append((off16, off16 + sum(tns) * P // 16))
    for c in range(C):
        for (a, b_) in h1cols:
            ncols = b_ - a
            pos = np.arange(ncols * 16)
            IDX[c, a:b_] = zrows[1][(pos + c * 7) % NZ].reshape(ncols, 16)

    loc_src = (lsrc_s - half_s * HB).astype(np.int16)
    off16_e = np.array([round_off[(h, j)] for h, j in
                        zip(half_s.tolist(), slot.tolist())], np.int64)
    col_e = off16_e + r_e // 16
    lane_e = r_e % 16
    IDX[c_e, col_e, lane_e] = loc_src
    IDX = np.tile(IDX.transpose(0, 2, 1), (1, 8, 1))

    pw16 = NLP // 16

    def tile_pt(vals_pad):
        """[C, NLP] in u order -> [C, P, T] (partition, tile)."""
        return np.ascontiguousarray(vals_pad.reshape(C, T, P).transpose(0, 2, 1))

    no_p = np.zeros((C, NLP), np.float32); no_p[:, :NL] = norm_out[ranked]
    ni_p = np.zeros((C, NLP), np.float32); ni_p[:, :NL] = norm_in[ranked]
    NO = tile_pt(no_p); NI = tile_pt(ni_p)

    def xbatches(rows):
        """[C, NLP, FIN] -> [C, NB, KSL, NB*KS*P] bf16 batched-transposed."""
        xr = rows.reshape(C, NB, NB, P, KS, KSL)    # c, b, tt, j, s, p
        return np.ascontiguousarray(
            xr.transpose(0, 1, 5, 2, 4, 3).reshape(C, NB, KSL, NB * KS * P)
        ).astype(ml_dtypes.bfloat16)

    xp = np.zeros((C, NLP, FIN), np.float32)
    xp[:, :NL] = x[ranked]
    XTB = xbatches(xp)
    # permuted view: host pre-gathers x[perm] rows for each core
    xp[:, :NL] = x[perm[ranked]]
    XPB = xbatches(xp)

    w1s = np.ascontiguousarray(W1.reshape(KS, KSL, H)).astype(ml_dtypes.bfloat16)
    rep = lambda v: np.ascontiguousarray(np.tile(v[None, :], (P, 1)))
    wv = Wm.sum(axis=1)
    wvb = np.concatenate([wv, [bm.sum()]]).astype(np.float32)
    hasb1 = bool(np.any(b1)); hasb2 = bool(np.any(b2))

    ck = (tuple(kcom.ravel().tolist()), hasb1, hasb2,
          os.environ.get("KERNEL_NPHASE", "9"))
    if ck not in _cache:
        _cache[ck] = _build(rounds, hasb1, hasb2, iw16, pw16)
    nc = _cache[ck]

    shared = {
        "w1_in": w1s, "w2_in": W2.astype(ml_dtypes.bfloat16),
        "b1_in": rep(b1), "a1_in": rep(a1),
        "b2_in": rep(b2), "a2_in": rep(a2), "wv_in": rep(wvb),
    }
    in_maps = []
    for c in range(C):
        m = dict(shared)
        m.update({
            "xtb": XTB[c], "xpb": XPB[c], "idx": IDX[c],
            "no_in": NO[c], "ni_in": NI[c],
        })
        in_maps.append(m)

    trace = os.environ.get("KERNEL_TRACE", "0") == "1"
    tmpdir = os.environ.get("KERNEL_TMPDIR") or None
    res = run_bass_kernel_spmd(nc, in_maps, core_ids=list(range(C)), trace=trace,
                               tmpdir=tmpdir)
    kernel.last_result = res
    if res.exec_time_ns is not None:
        print(f"HW exec time: {res.exec_time_ns} ns")
        kernel.last_exec_time_ns = res.exec_time_ns

    z = np.empty((2, N), np.float32)
    for c in range(C):
        o = res.results[c]["out"].reshape(2, P, T)
        for v in range(2):
            z[v, ranked[c]] = o[v].T.reshape(NLP)[:NL]
    return np.concatenate([z[0], z[1]]).astype(np.float32)


# revision 29
# speedup vs baseline: 1.4337x; 1.4337x over previous
"""DinkNet GNN (2-layer GraphConv encoder, two views) on 8 Trainium2 NeuronCores.

Strategy (self-contained; shapes hardcoded for the nn_DinkNet_dgl problem):
  - Nodes are sharded across 8 cores; per-core node u (sorted position) maps to
    table row c*NLP + (u%128)*T + (u//128) so per-partition slab writes are
    contiguous. Graph preprocessing (balance, rounds, index arrays) on host.
  - Math folding: (x*no) @ W1 = no * (x@W1); encode(x[perm]) reuses y = x@W1
    via row gathers; final (z@Wm+bm).sum(1) = h2 @ Wm.sum(1) + bm.sum().
  - The segment-sums over 800k edges run as batched GPSIMD dma_gather rounds
    (int16 indices; node space split in two halves so local indices fit).
    Both views interleave in one 512B table row (one gather feeds both views).
  - dma_gather descriptor emission on the Q7 pair is the bottleneck
    (~8ns/idx), so padding is minimized: src halves are balanced per-dst by
    local search, and nodes are tiled by max(c0,c1) so per-tile slot counts
    are tight. Padding indices cycle over 88 distinct zero rows to avoid
    serializing the HBM bank holding a single dummy row.
"""
import os
import numpy as np
import ml_dtypes

import concourse.bass as bass
import concourse.bacc as bacc
import concourse.mybir as mybir
import concourse.tile as tile
from concourse.bass_utils import run_bass_kernel_spmd
from concourse.masks import make_identity

# Problem shapes (hardcoded per contract).
N, E, FIN, H = 50000, 800000, 500, 96
C = 8                 # cores
P = 128               # partitions
NL = N // C           # 6250 real nodes per core
T = (NL + P - 1) // P # 49 rank tiles per core
NLP = T * P           # 6272 padded nodes per core
NTAB = C * NLP        # 50176 table rows
HB = NTAB // 2        # 25088: half boundary for int16 indices
EL = 128              # padded per-view row length (bf16 -> 256B)
EL2 = 2 * EL          # pair row: [view1 | view2] -> 512B in bf16
KS = 4                # k-slabs for the x @ W1 matmul (500 = 4*125)
KSL = FIN // KS       # 125
NB = 7                # tiles per batched x load (49 = 7*7)
RBCAP = T             # max pair-blocks per merged gather (SBUF budget)

F32 = mybir.dt.float32
BF16 = mybir.dt.bfloat16
TB = BF16
I16 = mybir.dt.int16
AOp = mybir.AluOpType

_cache = {}


def _bc_inner(ap, n):
    """Broadcast AP over a new innermost dim of size n (stride 0)."""
    return bass.AP(ap.tensor, ap.offset, list(ap.ap) + [[0, n]])


def _bc_middle(ap2d, n):
    """[128, F] AP -> [128, n, F] with the middle dim broadcast (stride 0)."""
    a = list(ap2d.ap)
    return bass.AP(ap2d.tensor, ap2d.offset, [a[0], [0, n], a[1]])


def _build(rounds, hasb1, hasb2, iw16, pw16, sim_mode=False):
    """rounds: list of (half, [tn...], off16) shared by both SpMM layers."""
    nc = bacc.Bacc(None, num_devices=1 if sim_mode else C,
                   target_bir_lowering=False, debug=False,
                   num_swdge_queues=4)

    # ---- inputs ----
    xtb = nc.dram_tensor("xtb", [NB, KSL, NB * KS * P], BF16, kind="ExternalInput")
    xpb = nc.dram_tensor("xpb", [NB, KSL, NB * KS * P], BF16, kind="ExternalInput")
    idx = nc.dram_tensor("idx", [P, iw16], I16, kind="ExternalInput")
    no_in = nc.dram_tensor("no_in", [P, T], F32, kind="ExternalInput")
    ni_in = nc.dram_tensor("ni_in", [P, T], F32, kind="ExternalInput")
    w1_in = nc.dram_tensor("w1_in", [KS, KSL, H], BF16, kind="ExternalInput")
    w2_in = nc.dram_tensor("w2_in", [H, H], BF16, kind="ExternalInput")
    b1_in = nc.dram_tensor("b1_in", [P, H], F32, kind="ExternalInput")
    a1_in = nc.dram_tensor("a1_in", [P, H], F32, kind="ExternalInput")
    b2_in = nc.dram_tensor("b2_in", [P, H], F32, kind="ExternalInput")
    a2_in = nc.dram_tensor("a2_in", [P, H], F32, kind="ExternalInput")
    wv_in = nc.dram_tensor("wv_in", [P, H + 1], F32, kind="ExternalInput")
    out = nc.dram_tensor("out", [2, NLP], F32, kind="ExternalOutput")

    nphase = int(os.environ.get("KERNEL_NPHASE", "9"))

    with tile.TileContext(nc) as tc:
        with (
            tc.tile_pool(name="cst", bufs=1) as cst,
            tc.tile_pool(name="wrk", bufs=2) as wrk,
            tc.tile_pool(name="ps", bufs=2, space="PSUM") as ps,
            tc.tile_pool(name="dr", bufs=1, space="DRAM") as dr,
        ):
            # ---- resident constants ----
            idxt = cst.tile([P, iw16], I16)
            nc.sync.dma_start(out=idxt[:], in_=idx[:])
            no_t = cst.tile([P, T], F32)
            nc.sync.dma_start(out=no_t[:], in_=no_in[:])
            ni_t = cst.tile([P, T], F32)
            nc.sync.dma_start(out=ni_t[:], in_=ni_in[:])
            w1t = cst.tile([KSL, KS * H], BF16)
            nc.sync.dma_start(
                out=w1t[:].rearrange("p (s h) -> p s h", s=KS),
                in_=w1_in[:].rearrange("s p h -> p s h"),
            )
            w2t = cst.tile([H, H], BF16)
            nc.sync.dma_start(out=w2t[:], in_=w2_in[:])
            b1r = cst.tile([P, H], F32)
            nc.sync.dma_start(out=b1r[:], in_=b1_in[:])
            a1r = cst.tile([P, H], F32)
            nc.sync.dma_start(out=a1r[:], in_=a1_in[:])
            b2r = cst.tile([P, H], F32)
            nc.sync.dma_start(out=b2r[:], in_=b2_in[:])
            a2r = cst.tile([P, H], F32)
            nc.sync.dma_start(out=a2r[:], in_=a2_in[:])
            wvr = cst.tile([P, H + 1], F32)
            nc.sync.dma_start(out=wvr[:], in_=wv_in[:])
            ident = cst.tile([P, P], F32)
            make_identity(nc, ident[:])
            identb = cst.tile([P, P], TB)
            make_identity(nc, identb[:])

            # Pool touches: advance Pool's vector clock past the const loads
            tch = cst.tile([1, 8], I16, name="tch")
            nc.gpsimd.dma_start(out=tch[0:1, 0:4], in_=idxt[0:1, 0:4])

            # ---- DRAM tables ----
            ypair = dr.tile([NTAB, EL2], TB, addr_space="Shared", name="ypair")
            gpair = dr.tile([NTAB, EL2], TB, addr_space="Shared", name="gpair")
            psl = dr.tile([NLP, EL2], TB, name="psl")
            gsl = dr.tile([NLP, EL2], TB, name="gsl")

            tchf = cst.tile([1, 4], TB, name="tchf")

            def allgather(sl, tab):
                if sim_mode:
                    nc.sync.dma_start(out=tab[0:NLP, :], in_=sl[:])
                else:
                    nc.gpsimd.collective_compute(
                        "AllGather", AOp.bypass,
                        replica_groups=[list(range(C))],
                        ins=[sl[:].opt()], outs=[tab[:].opt()],
                    )
                nc.gpsimd.dma_start(out=tchf[:], in_=tab[0:1, 0:4])

            # ---- phase 1: both views y_v = no * (x_v @ W1), v in {orig, perm} ----
            # pstage holds pair rows at EL2 pitch (pad cols pre-zeroed) so the
            # psl slab write is one per-partition-contiguous DMA. The permuted
            # view's x rows are pre-gathered on the host (perm is host data).
            pstage = wrk.tile([P, T * EL2], TB, tag="pstage", bufs=1)
            nc.vector.memset(pstage[:], 0.0)
            for b in range(NB):
                xtile = wrk.tile([KSL, NB * KS * P], BF16, tag="xtile", bufs=2)
                nc.sync.dma_start(out=xtile[:], in_=xtb[b])
                xptile = wrk.tile([KSL, NB * KS * P], BF16, tag="xtile", bufs=2)
                nc.sync.dma_start(out=xptile[:], in_=xpb[b])
                # 4 PSUM banks per view batch -> one strided DVE drain each
                for t0 in range(0, NB, 4):
                    ntt = min(4, NB - t0)
                    for v, xt_ in enumerate((xtile, xptile)):
                        yps = ps.tile([P, 4 * H], F32, tag="bank", bufs=8, space="PSUM")
                        for q in range(ntt):
                            tt = t0 + q
                            for s in range(KS):
                                nc.tensor.matmul(
                                    out=yps[:, q * H:(q + 1) * H],
                                    lhsT=xt_[:, (tt * KS + s) * P:(tt * KS + s + 1) * P],
                                    rhs=w1t[:, s * H:(s + 1) * H],
                                    start=(s == 0), stop=(s == KS - 1),
                                )
                        t = b * NB + t0
                        ydst = bass.AP(pstage.tensor,
                                       pstage.offset + t * EL2 + v * EL,
                                       [list(pstage.ap[0]), [EL2, ntt], [1, H]])
                        nc.vector.tensor_tensor(
                            out=ydst,
                            in0=yps[:, :ntt * H].rearrange("p (n h) -> p n h", h=H),
                            in1=_bc_inner(no_t[:, t:t + ntt], H), op=AOp.mult,
                        )
            if nphase >= 3:
                nc.sync.dma_start(
                    out=bass.AP(psl.tensor, psl.offset, [[T * EL2, P], [1, T * EL2]]),
                    in_=pstage[:],
                )
                allgather(psl, ypair)

            # ---- SpMM over a pair table; both views accumulated ----
            # Tiles 0..TP-1 accumulate on the TensorEngine (identity matmul
            # into PSUM, bank-chunked 5 tiles each); tiles TP.. on DVE.
            TP = 20
            NCHUNK = TP // 5
            # flat sub-round list to place start/stop flags per PSUM chunk
            subs = []
            for (h, tns, off16) in rounds:
                for si, tn in enumerate(tns):
                    subs.append(tn)
            last_cover = {}
            for k in range(NCHUNK):
                last_cover[k] = max(i for i, tn in enumerate(subs) if tn > 5 * k)

            def spmm_pair(tab, layer, br, ar, hasb):
                accs = []
                for v in range(2):
                    acc = wrk.tile([P, T * H], F32, tag=f"acc{v}", bufs=1,
                                   name=f"acc{v}_{layer}")
                    nc.vector.memset(acc[:, TP * H:], 0.0)
                    accs.append(acc)
                accps = [[ps.tile([P, 5 * H], F32, tag="bank",
                                  space="PSUM", bufs=8, name=f"accps{v}_{k}_{layer}")
                          for k in range(NCHUNK)] for v in range(2)]
                si = 0
                for ri, (h, tns, off16) in enumerate(rounds):
                    ntot = sum(tns)
                    rb = wrk.tile([P, RBCAP * EL2], TB, tag="rb2", bufs=3,
                                  name=f"rb_{layer}_{h}_{off16}")
                    src = tab[0:HB, :] if h == 0 else tab[HB:NTAB, :]
                    nc.gpsimd.dma_gather(
                        out_ap=rb[:, :ntot * EL2].rearrange("p (b e) -> p b e", e=EL2),
                        in_ap=src,
                        idxs_ap=idxt[:, off16:off16 + ntot * P // 16],
                        num_idxs=ntot * P, num_idxs_reg=ntot * P, elem_size=EL2,
                        single_packet=False, queue_num=ri % 3,
                    )
                    boff = 0
                    for tn in tns:
                        for v in range(2):
                            # PE part: tiles 0..min(tn,TP)
                            for k in range(NCHUNK):
                                nb = min(tn, TP) - 5 * k
                                if nb <= 0:
                                    break
                                nb = min(nb, 5)
                                r3 = bass.AP(
                                    rb.tensor,
                                    rb.offset + (boff + 5 * k) * EL2 + v * EL,
                                    [list(rb.ap[0]), [EL2, nb], [1, H]],
                                )
                                nc.tensor.matmul(
                                    out=accps[v][k][:, :nb * H],
                                    lhsT=identb[:], rhs=r3,
                                    start=(si == 0), stop=(si == last_cover[k]),
                                )
                            # DVE tail: tiles TP..tn
                            if tn > TP:
                                a3 = bass.AP(
                                    accs[v].tensor,
                                    accs[v].offset + TP * H,
                                    [list(accs[v].ap[0]), [H, tn - TP], [1, H]],
                                )
                                r3t = bass.AP(
                                    rb.tensor,
                                    rb.offset + (boff + TP) * EL2 + v * EL,
                                    [list(rb.ap[0]), [EL2, tn - TP], [1, H]],
                                )
                                nc.vector.tensor_tensor(out=a3, in0=a3, in1=r3t, op=AOp.add)
                        boff += tn
                        si += 1
                # drain PSUM chunks into acc (add onto memset-zero region)
                for v in range(2):
                    for k in range(NCHUNK):
                        nc.vector.tensor_copy(
                            out=accs[v][:, 5 * k * H:(5 * k + 5) * H],
                            in_=accps[v][k][:],
                        )
                # postops per view (bf16 for 2x DVE rate; acc stays f32)
                outs_v = []
                for v in range(2):
                    acc = accs[v]
                    a3 = acc[:].rearrange("p (t h) -> p t h", h=H)
                    hb = wrk.tile([P, T * H], TB, tag=f"hb{v}", bufs=1,
                                  name=f"hb_{layer}_{v}")
                    h3 = hb[:].rearrange("p (t h) -> p t h", h=H)
                    nc.vector.tensor_tensor(out=h3, in0=a3, in1=_bc_inner(ni_t[:], H), op=AOp.mult)
                    if hasb:
                        nc.vector.tensor_tensor(out=h3, in0=h3, in1=_bc_middle(br[:], T), op=AOp.add)
                    pos = wrk.tile([P, T * H], TB, tag="pos", bufs=2,
                                   name=f"pos_{layer}_{v}")
                    nc.vector.tensor_scalar(out=pos[:], in0=hb[:], scalar1=0.0, scalar2=None, op0=AOp.max)
                    nc.vector.tensor_scalar(out=hb[:], in0=hb[:], scalar1=0.0, scalar2=None, op0=AOp.min)
                    nc.vector.tensor_tensor(out=h3, in0=h3, in1=_bc_middle(ar[:], T), op=AOp.mult)
                    nc.vector.tensor_tensor(out=hb[:], in0=hb[:], in1=pos[:], op=AOp.add)
                    if layer == 1:
                        nc.vector.tensor_tensor(out=h3, in0=h3, in1=_bc_inner(no_t[:], H), op=AOp.mult)
                        gst = wrk.tile([P, T * EL2], TB, tag="pstage", bufs=1,
                                       name="gstage") if v == 0 else outs_v[0]
                        if v == 0:
                            nc.vector.memset(gst[:], 0.0)
                        # batched: 4 transposes into one PSUM bank, one copy,
                        # 4 matmuls into one bank, one strided copy out
                        for t0 in range(0, T, 4):
                            nt = min(4, T - t0)
                            tp = ps.tile([H, 4 * P], TB, tag="bank", bufs=8, space="PSUM")
                            for q in range(nt):
                                nc.tensor.transpose(
                                    out=tp[:, q * P:(q + 1) * P],
                                    in_=hb[:, (t0 + q) * H:(t0 + q + 1) * H],
                                    identity=identb[:],
                                )
                            gsT = wrk.tile([H, 4 * P], TB, tag="gsT", bufs=3)
                            nc.vector.tensor_copy(out=gsT[:, :nt * P], in_=tp[:, :nt * P])
                            gp = ps.tile([P, 4 * H], F32, tag="bank", bufs=8, space="PSUM")
                            for q in range(nt):
                                nc.tensor.matmul(out=gp[:, q * H:(q + 1) * H],
                                                 lhsT=gsT[:, q * P:(q + 1) * P],
                                                 rhs=w2t[:], start=True, stop=True)
                            gdst = bass.AP(gst.tensor, gst.offset + t0 * EL2 + v * EL,
                                           [list(gst.ap[0]), [EL2, nt], [1, H]])
                            nc.vector.tensor_copy(
                                out=gdst,
                                in_=gp[:, :nt * H].rearrange("p (n h) -> p n h", h=H))
                        outs_v.append(gst)
                    else:
                        zb = wrk.tile([P, T], F32, tag="zb", bufs=2, name=f"zb_{v}")
                        nc.vector.tensor_tensor(out=h3, in0=h3,
                                                in1=_bc_middle(wvr[:, 0:H], T), op=AOp.mult)
                        nc.vector.tensor_reduce(
                            out=zb[:], in_=h3, axis=mybir.AxisListType.X, op=AOp.add,
                        )
                        nc.vector.tensor_scalar(
                            out=zb[:], in0=zb[:], scalar1=wvr[:, H:H + 1],
                            scalar2=None, op0=AOp.add,
                        )
                        nc.sync.dma_start(
                            out=bass.AP(out, v * NLP, [[T, P], [1, T]]), in_=zb[:],
                        )
                        outs_v.append(None)
                return outs_v

            if nphase >= 4:
                g1, g2 = spmm_pair(ypair, 1, b1r, a1r, hasb1)
                nc.sync.dma_start(
                    out=bass.AP(gsl.tensor, gsl.offset, [[T * EL2, P], [1, T * EL2]]),
                    in_=g2[:],
                )
                allgather(gsl, gpair)
            if nphase >= 5:
                spmm_pair(gpair, 2, b2r, a2r, hasb2)

    nc.finalize()
    return nc


def _wrap16(vals):
    """[n] int array -> wrapped [128, n/16] int16 (replicated across 8 Q7 cores)."""
    n = vals.shape[-1]
    assert n % 16 == 0
    w = vals.reshape(n // 16, 16).T.astype(np.int16)
    return np.tile(w, (8, 1))


def _balance_halves(src, dst, scnt, sstart, dst_by_src):
    """Greedy + local-search 2-coloring of src nodes so each dst's in-edges
    split evenly between halves. Returns half_of (int8[N])."""
    cnt_diff = np.zeros(N, np.int32)
    half_of = np.zeros(N, np.int8)
    cap = [N // 2, N // 2]
    order = np.argsort(-scnt, kind="stable")
    for s_ in order:
        D = dst_by_src[sstart[s_]:sstart[s_ + 1]]
        sd = cnt_diff[D].sum() if len(D) else (cap[1] - cap[0])
        h = 0 if sd < 0 else 1
        if cap[h] == 0:
            h = 1 - h
        half_of[s_] = h
        cap[h] -= 1
        if len(D):
            cnt_diff[D] += 1 - 2 * h

    sizes = np.bincount(half_of, minlength=2)
    for sweep in range(6):
        nflip = 0
        order = np.random.RandomState(sweep).permutation(N)
        for s_ in order:
            b, e_ = sstart[s_], sstart[s_ + 1]
            if b == e_:
                continue
            D = dst_by_src[b:e_]
            sd = int(cnt_diff[D].sum())
            k = e_ - b
            h = half_of[s_]
            if h == 0 and sd > k and sizes[1] < N // 2 + 64:
                half_of[s_] = 1; cnt_diff[D] -= 2
                sizes[0] -= 1; sizes[1] += 1; nflip += 1
            elif h == 1 and -sd > k and sizes[0] < N // 2 + 64:
                half_of[s_] = 0; cnt_diff[D] += 2
                sizes[1] -= 1; sizes[0] += 1; nflip += 1
        if nflip < 50:
            break
    imbal = int(sizes[0]) - N // 2
    if imbal != 0:
        h_from = 0 if imbal > 0 else 1
        cands = np.where(half_of == h_from)[0]
        damage = np.zeros(len(cands))
        for i, s_ in enumerate(cands):
            D = dst_by_src[sstart[s_]:sstart[s_ + 1]]
            sd = int(cnt_diff[D].sum())
            k = len(D)
            damage[i] = (4 * k - 4 * sd) if h_from == 0 else (4 * k + 4 * sd)
        for s_ in cands[np.argsort(damage)[:abs(imbal)]]:
            D = dst_by_src[sstart[s_]:sstart[s_ + 1]]
            half_of[s_] = 1 - h_from
            cnt_diff[D] += -2 if h_from == 0 else 2
    return half_of


def kernel(x, src, dst, perm, W1, b1, a1, W2, b2, a2, Wm, bm):
    x = np.ascontiguousarray(np.asarray(x, np.float32))
    src = np.asarray(src, np.int64)
    dst = np.asarray(dst, np.int64)
    perm = np.asarray(perm, np.int64)
    W1 = np.asarray(W1, np.float32); W2 = np.asarray(W2, np.float32)
    Wm = np.asarray(Wm, np.float32)
    b1 = np.asarray(b1, np.float32); b2 = np.asarray(b2, np.float32)
    a1 = np.asarray(a1, np.float32); a2 = np.asarray(a2, np.float32)
    bm = np.asarray(bm, np.float32)

    deg_out = np.bincount(src, minlength=N)
    deg_in = np.bincount(dst, minlength=N)
    norm_out = np.maximum(deg_out, 1).astype(np.float32) ** -0.5
    norm_in = np.maximum(deg_in, 1).astype(np.float32) ** -0.5

    eo = np.argsort(src, kind="stable")
    dst_by_src = dst[eo]
    scnt = np.bincount(src, minlength=N)
    sstart = np.concatenate(([0], np.cumsum(scnt)))
    half_of = _balance_halves(src, dst, scnt, sstart, dst_by_src)

    # per-half in-edge counts of every dst
    c0 = np.bincount(dst[half_of[src] == 0], minlength=N)
    c1 = np.bincount(dst[half_of[src] == 1], minlength=N)
    cmax = np.maximum(c0, c1); cmin = np.minimum(c0, c1)

    # per-half sort by max(c0,c1) so tiles have tight slot counts; stripe
    # over 4 cores each. ranked[c] lists nodes in position order u; the table
    # row of position u is (u%128)*T + u//128 (partition-contiguous slabs).
    ranked = np.empty((C, NL), np.int64)
    Lmap = np.empty(N, np.int64)   # node -> table row
    Umap = np.empty(N, np.int64)   # node -> core-local position u
    Cmap = np.empty(N, np.int64)   # node -> core
    for h in range(2):
        ids = np.where(half_of == h)[0]
        ids = ids[np.lexsort((-cmin[ids], -cmax[ids]))]
        for cc in range(4):
            c = h * 4 + cc
            rid = ids[cc::4]
            ranked[c] = rid
            u = np.arange(NL)
            Umap[rid] = u
            Cmap[rid] = c
            Lmap[rid] = c * NLP + (u % P) * T + (u // P)

    lsrc = Lmap[src]
    half = (lsrc >= HB).astype(np.int64)

    # dst-side positions in u space
    udst = Umap[dst]
    cdst = Cmap[dst]
    key = (cdst * NLP + udst) * 2 + half
    es = np.argsort(key, kind="stable")
    key_s = key[es]
    cnt = np.bincount(key_s, minlength=2 * NTAB)
    starts = np.concatenate(([0], np.cumsum(cnt)))[:-1]
    slot = np.arange(E) - starts[key_s]
    lsrc_s = lsrc[es]
    half_s = half[es]

    c_e = cdst[es]
    r_e = udst[es]          # position within core (u space)
    t_e = r_e // P
    kh = np.zeros((2, C, T), np.int64)
    np.maximum.at(kh, (half_s, c_e, t_e), slot + 1)
    kcom = kh.max(axis=1)
    kcom = np.maximum.accumulate(kcom[:, ::-1], axis=1)[:, ::-1]

    # merged rounds, capped at RBCAP pair-blocks per gather
    rounds = []
    col16 = 0
    round_off = {}
    for h in range(2):
        j = 0
        kmax = int(kcom[h, 0])
        while j < kmax:
            tns = []
            while j < kmax:
                tn = int((kcom[h] > j).sum())
                if tns and sum(tns) + tn > RBCAP:
                    break
                round_off[(h, j)] = col16 + sum(tns) * P // 16
                tns.append(tn)
                j += 1
            rounds.append((h, tns, col16))
            col16 += sum(tns) * P // 16
    iw16 = col16

    # padding indices cycle over the 88 zero rows of each half (distinct HBM
    # addresses -> no single-bank serialization). Zero rows per half: cores'
    # positions u in [NL, NLP) -> rows (u%128)*T + 48.
    zrows = np.empty((2, 4 * (NLP - NL)), np.int64)
    for h in range(2):
        rows = []
        for u in range(NL, NLP):
            for cc in range(4):
                rows.append(cc * NLP + (u % P) * T + (u // P))
        zrows[h] = np.array(rows, np.int64)
    NZ = zrows.shape[1]

    IDX = np.empty((C, iw16, 16), np.int16)
    for c in range(C):
        # fill with cycling zero-row pattern per half section
        fill = np.empty(iw16 * 16, np.int64)
        pos = np.arange(iw16 * 16)
        fill[:] = zrows[0][(pos + c * 7) % NZ]
        IDX[c] = fill.reshape(iw16, 16)
    # overwrite half-1 sections with half-1 zero rows
    h1cols = []
    for (h, tns, off16) in rounds:
        if h == 1:
            h1cols.append((off16, off16 + sum(tns) * P // 16))
    for c in range(C):
        for (a, b_) in h1cols:
            ncols = b_ - a
            pos = np.arange(ncols * 16)
            IDX[c, a:b_] = zrows[1][(pos + c * 7) % NZ].reshape(ncols, 16)

    loc_src = (lsrc_s - half_s * HB).astype(np.int16)
    off16_e = np.array([round_off[(h, j)] for h, j in
                        zip(half_s.tolist(), slot.tolist())], np.int64)
    col_e = off16_e + r_e // 16
    lane_e = r_e % 16
    IDX[c_e, col_e, lane_e] = loc_src
    IDX = np.tile(IDX.transpose(0, 2, 1), (1, 8, 1))

    pw16 = NLP // 16

    def tile_pt(vals_pad):
        """[C, NLP] in u order -> [C, P, T] (partition, tile)."""
        return np.ascontiguousarray(vals_pad.reshape(C, T, P).transpose(0, 2, 1))

    no_p = np.zeros((C, NLP), np.float32); no_p[:, :NL] = norm_out[ranked]
    ni_p = np.zeros((C, NLP), np.float32); ni_p[:, :NL] = norm_in[ranked]
    NO = tile_pt(no_p); NI = tile_pt(ni_p)

    def xbatches(rows):
        """[C, NLP, FIN] -> [C, NB, KSL, NB*KS*P] bf16 batched-transposed."""
        xr = rows.reshape(C, NB, NB, P, KS, KSL)    # c, b, tt, j, s, p
        return np.ascontiguousarray(
            xr.transpose(0, 1, 5, 2, 4, 3).reshape(C, NB, KSL, NB * KS * P)
        ).astype(ml_dtypes.bfloat16)

    xp = np.zeros((C, NLP, FIN), np.float32)
    xp[:, :NL] = x[ranked]
    XTB = xbatches(xp)
    # permuted view: host pre-gathers x[perm] rows for each core
    xp[:, :NL] = x[perm[ranked]]
    XPB = xbatches(xp)

    w1s = np.ascontiguousarray(W1.reshape(KS, KSL, H)).astype(ml_dtypes.bfloat16)
    rep = lambda v: np.ascontiguousarray(np.tile(v[None, :], (P, 1)))
    wv = Wm.sum(axis=1)
    wvb = np.concatenate([wv, [bm.sum()]]).astype(np.float32)
    hasb1 = bool(np.any(b1)); hasb2 = bool(np.any(b2))

    ck = (tuple(kcom.ravel().tolist()), hasb1, hasb2,
          os.environ.get("KERNEL_NPHASE", "9"))
    if ck not in _cache:
        _cache[ck] = _build(rounds, hasb1, hasb2, iw16, pw16)
    nc = _cache[ck]

    shared = {
        "w1_in": w1s, "w2_in": W2.astype(ml_dtypes.bfloat16),
        "b1_in": rep(b1), "a1_in": rep(a1),
        "b2_in": rep(b2), "a2_in": rep(a2), "wv_in": rep(wvb),
    }
    in_maps = []
    for c in range(C):
        m = dict(shared)
        m.update({
            "xtb": XTB[c], "xpb": XPB[c], "idx": IDX[c],
            "no_in": NO[c], "ni_in": NI[c],
        })
        in_maps.append(m)

    trace = os.environ.get("KERNEL_TRACE", "0") == "1"
    tmpdir = os.environ.get("KERNEL_TMPDIR") or None
    res = run_bass_kernel_spmd(nc, in_maps, core_ids=list(range(C)), trace=trace,
                               tmpdir=tmpdir)
    kernel.last_result = res
    if res.exec_time_ns is not None:
        print(f"HW exec time: {res.exec_time_ns} ns")
        kernel.last_exec_time_ns = res.exec_time_ns

    z = np.empty((2, N), np.float32)
    for c in range(C):
        o = res.results[c]["out"].reshape(2, P, T)
        for v in range(2):
            z[v, ranked[c]] = o[v].T.reshape(NLP)[:NL]
    return np.concatenate([z[0], z[1]]).astype(np.float32)
